# revision 2
# baseline (speedup 1.0000x reference)
"""Trainium2 Bass kernel for batched displacement-operator construction.

Math: for each alpha_b,
    Da[b] = diag(u) @ (V @ diag(exp(-i r lam)) @ V.T) @ diag(v)
with u_i = w^i, v_j = (1/w)^j, w = i*alpha/|alpha|.  Since u_i*v_j = w^(i-j)
(|w| == 1 up to fp eps), the outer phase factor is a Toeplitz matrix whose
tiles are slices of a per-alpha [128, 1920] shifted-window table, precomputed
on the host.  On device per alpha: 2 real 1024^3 matmuls (cos and -sin parts,
float32r for full-rate fp32 on the PE), then a complex elementwise multiply
by the phase tiles (4 muls on DVE reading PSUM, 2 add/sub on GPSIMD).

Sharding: 16 alphas data-parallel over 8 cores (2 per core); evecs replicated.
"""

import sys

sys.path.insert(0, "/opt/trn_rl_repo")

import numpy as np

N = 1024
B = 16
NCORES = 8
APC = B // NCORES  # alphas per core
P = 128
KC = N // P  # contraction chunks
MC = N // P  # output row chunks
NT = 512  # matmul free-dim tile (fp32 PSUM bank)
NNT = N // NT  # output col chunks
WWIN = 1920  # phase-window free size
C0 = 896  # phase-window offset constant

_cache = {}


def _build_module(reps=1):
    import contextlib

    import concourse.bacc as bacc
    import concourse.mybir as mybir
    import concourse.tile as tile

    f32 = mybir.dt.float32
    f32r = mybir.dt.float32r
    Alu = mybir.AluOpType
    Act = mybir.ActivationFunctionType

    nc = bacc.Bacc(
        "TRN2",
        target_bir_lowering=False,
        debug=False,
        num_devices=NCORES,
    )

    vt_d = nc.dram_tensor("vt", [N, N], f32, kind="ExternalInput")
    esc_d = nc.dram_tensor("esc", [P, APC * 2 * KC], f32, kind="ExternalInput")
    ph_d = nc.dram_tensor("ph", [P, APC * 2 * WWIN], f32, kind="ExternalInput")
    outr_d = nc.dram_tensor("outr", [APC, N, N], f32, kind="ExternalOutput")
    outi_d = nc.dram_tensor("outi", [APC, N, N], f32, kind="ExternalOutput")

    with tile.TileContext(nc) as tc:
        with (
            tc.tile_pool(name="const", bufs=1) as cpool,
            tc.tile_pool(name="wts", bufs=1) as wpool,
            tc.tile_pool(name="work", bufs=3) as work,
            tc.tile_pool(name="outp", bufs=3) as outp,
            tc.tile_pool(name="psum", bufs=2, space="PSUM") as pp,
        ):
            esc = cpool.tile([P, APC * 2 * KC], f32)
            ph = cpool.tile([P, APC * 2 * WWIN], f32)
            from concourse.masks import make_identity

            ident = cpool.tile([P, P], f32, name="ident")
            make_identity(nc, ident)

            # Per-chunk tiles so Tile tracks dependencies at chunk
            # granularity: the next alpha's weight scaling can overlap the
            # previous alpha's tail matmuls instead of waiting for them all.
            vt = [
                cpool.tile([P, N], f32r, tag=f"vt{kc}", name=f"vt{kc}")
                for kc in range(KC)
            ]
            lc = [
                wpool.tile([P, N], f32r, tag=f"lc{kc}", name=f"lc{kc}")
                for kc in range(KC)
            ]
            ls = [
                wpool.tile([P, N], f32r, tag=f"ls{kc}", name=f"ls{kc}")
                for kc in range(KC)
            ]

            # esc first (tiny, gates all weight scaling).  Split the vt
            # chunk loads between the HWDGE (sync) and SWDGE (gpsimd)
            # queues so they stream in parallel; ph goes last on SWDGE
            # since the phase tiles are first consumed much later.
            nc.gpsimd.dma_start(esc[:], esc_d[:])
            nc.gpsimd.dma_start(ph[:], ph_d[:])
            # The fp32r DRAM-input binding path crashes the exec unit, so
            # DMA fp32 and round to fp32r on-device (DVE cast producer).
            for kc in range(KC):
                tmp = work.tile([P, N], f32, tag="vtin")
                nc.sync.dma_start(tmp[:], vt_d[kc * P : (kc + 1) * P, :])
                nc.vector.tensor_copy(vt[kc][:], tmp[:])

            rep_ctx = (
                tc.For_i(0, reps, 1) if reps > 1 else contextlib.nullcontext()
            )
            with rep_ctx:
                _emit_body(nc, tc, vt, esc, ph, lc, ls, work, outp, pp,
                           outr_d, outi_d, mybir, wpool, ident)

    nc.compile()
    return nc


def _emit_body(nc, tc, vt, esc, ph, lc, ls, work, outp, pp, outr_d, outi_d,
               mybir, wpool, ident):
    f32 = mybir.dt.float32
    Alu = mybir.AluOpType
    Act = mybir.ActivationFunctionType
    HM = MC // 2  # mirror boundary: tiles (m>=HM, n=0) come from transposes
    if True:
            for a in range(APC):
                # Scale VT rows by er = cos(r*lam) and ei = -sin(r*lam)
                # (per-partition scalars) to form the matmul weights.
                for kc in range(KC):
                    col_er = a * 2 * KC + kc
                    col_ei = a * 2 * KC + KC + kc
                    # Split the scaling between ACT and DVE so neither is a
                    # serial bottleneck ahead of the matmuls.
                    nc.scalar.activation(
                        lc[kc][:], vt[kc][:], Act.Copy,
                        scale=esc[:, col_er : col_er + 1],
                    )
                    nc.vector.tensor_scalar_mul(
                        ls[kc][:], vt[kc][:], esc[:, col_ei : col_ei + 1]
                    )

                base_c = (a * 2) * WWIN
                base_s = (a * 2 + 1) * WWIN

                ev = {}
                for m in range(MC):
                    pc0 = pp.tile([P, NT], f32, tag="pc0")
                    pc1 = pp.tile([P, NT], f32, tag="pc1")
                    ps0 = pp.tile([P, NT], f32, tag="ps0")
                    ps1 = pp.tile([P, NT], f32, tag="ps1")
                    # C = V diag(er) V^T is symmetric: compute the n=1 column
                    # always, but for m >= HM build the n=0 tile by PE-
                    # transposing the earlier (m' < HM, n=1) tiles instead of
                    # an 8-deep matmul accumulation (64 MMs -> 32 transposes
                    # per alpha).  The UNSCALED vt block is the stationary
                    # operand so one fp32r weight load serves all streams.
                    for kc in range(KC):
                        wap = vt[kc][:, m * P : (m + 1) * P]
                        st = kc == 0
                        sp = kc == KC - 1
                        if m < HM:
                            nc.tensor.matmul(pc0[:], wap, lc[kc][:, 0:NT],
                                             start=st, stop=sp)
                            nc.tensor.matmul(ps0[:], wap, ls[kc][:, 0:NT],
                                             start=st, stop=sp)
                        nc.tensor.matmul(pc1[:], wap, lc[kc][:, NT:N],
                                         start=st, stop=sp)
                        nc.tensor.matmul(ps1[:], wap, ls[kc][:, NT:N],
                                         start=st, stop=sp)
                    if m < HM:
                        # Keep an SBUF copy of the n=1 tiles for the mirror
                        # transposes later (ACT has slack).
                        evc = wpool.tile([P, NT], f32, tag=f"evc{m}",
                                         name=f"evc{m}_{a}")
                        evs = wpool.tile([P, NT], f32, tag=f"evs{m}",
                                         name=f"evs{m}_{a}")
                        nc.scalar.activation(evc[:], pc1[:], Act.Copy)
                        nc.scalar.activation(evs[:], ps1[:], Act.Copy)
                        ev[m] = (evc, evs)
                    else:
                        q = m - HM
                        for mp in range(HM):
                            evc, evs = ev[mp]
                            nc.tensor.matmul(
                                pc0[:, mp * P : (mp + 1) * P],
                                evc[:, q * P : (q + 1) * P], ident[:],
                                is_transpose=True, start=True, stop=True,
                            )
                            nc.tensor.matmul(
                                ps0[:, mp * P : (mp + 1) * P],
                                evs[:, q * P : (q + 1) * P], ident[:],
                                is_transpose=True, start=True, stop=True,
                            )
                    for n in range(NNT):
                        pc = pc0 if n == 0 else pc1
                        ps = ps0 if n == 0 else ps1
                        t0 = C0 - P * m + NT * n
                        pr = ph[:, base_c + t0 : base_c + t0 + NT]
                        pi = ph[:, base_s + t0 : base_s + t0 + NT]
                        m1 = work.tile([P, NT], f32, tag="m1")
                        m2 = work.tile([P, NT], f32, tag="m2")
                        m3 = work.tile([P, NT], f32, tag="m3")
                        m4 = work.tile([P, NT], f32, tag="m4")
                        nc.vector.tensor_tensor(m1[:], pc[:], pr, Alu.mult)
                        nc.vector.tensor_tensor(m2[:], ps[:], pi, Alu.mult)
                        nc.vector.tensor_tensor(m3[:], pc[:], pi, Alu.mult)
                        nc.vector.tensor_tensor(m4[:], ps[:], pr, Alu.mult)
                        dar = outp.tile([P, NT], f32, tag="dar")
                        dai = outp.tile([P, NT], f32, tag="dai")
                        nc.gpsimd.tensor_tensor(dar[:], m1[:], m2[:], Alu.subtract)
                        nc.gpsimd.tensor_tensor(dai[:], m3[:], m4[:], Alu.add)
                        nc.sync.dma_start(
                            outr_d[a, m * P : (m + 1) * P, n * NT : (n + 1) * NT],
                            dar[:],
                        )
                        nc.sync.dma_start(
                            outi_d[a, m * P : (m + 1) * P, n * NT : (n + 1) * NT],
                            dai[:],
                        )


def _get_module():
    if "nc" not in _cache:
        _cache["nc"] = _build_module()
    return _cache["nc"]


def _host_precompute(alpha_real, alpha_imag, evals):
    """Per-alpha scalar tables, mirroring the reference's fp32 arithmetic."""
    ar = np.asarray(alpha_real, np.float32)
    ai = np.asarray(alpha_imag, np.float32)
    ev = np.asarray(evals, np.float32)

    esc_all = np.empty((B, 2, KC, P), np.float32)  # (b, er/ei, kc, p)
    ph_all = np.empty((B, 2, P, WWIN), np.float32)  # (b, re/im, p, w)

    prow = np.arange(P)[:, None]
    scol = np.arange(WWIN)[None, :]
    idx = (prow - scol) + C0 + (N - 1)  # into d-table of length 2N-1

    for b in range(B):
        alpha = np.complex64(complex(ar[b], ai[b]))
        r = np.float32(np.abs(alpha)) + np.float32(1e-10)
        eit = np.complex64(alpha / r)
        w = np.complex128(1j) * np.complex128(eit)

        t32 = (np.float32(r) * ev).astype(np.float32)
        t64 = t32.astype(np.float64)
        er = np.cos(t64).astype(np.float32)
        ei = (-np.sin(t64)).astype(np.float32)
        esc_all[b, 0] = er.reshape(KC, P)
        esc_all[b, 1] = ei.reshape(KC, P)

        d = np.arange(-(N - 1), N)
        ptab = w ** d  # complex128, |w|~1 so no overflow
        wc = ptab.real.astype(np.float32)
        ws = ptab.imag.astype(np.float32)
        ph_all[b, 0] = wc[idx]
        ph_all[b, 1] = ws[idx]

    return esc_all, ph_all


def _make_in_maps(alpha_real, alpha_imag, evals, evecs):
    evecs_f = np.ascontiguousarray(np.asarray(evecs, np.float32))
    vt_np = np.ascontiguousarray(evecs_f.T)
    esc_all, ph_all = _host_precompute(alpha_real, alpha_imag, evals)

    in_maps = []
    for c in range(NCORES):
        bs = [c * APC + a for a in range(APC)]
        # esc columns: per alpha [er cols | ei cols]; value at (p, col) with
        # col = a*2*KC + which*KC + kc  ->  esc_all[b, which, kc, p]
        esc = np.empty((P, APC * 2 * KC), np.float32)
        ph = np.empty((P, APC * 2 * WWIN), np.float32)
        for a, b in enumerate(bs):
            for which in range(2):
                cols = a * 2 * KC + which * KC
                esc[:, cols : cols + KC] = esc_all[b, which].T
                wbase = (a * 2 + which) * WWIN
                ph[:, wbase : wbase + WWIN] = ph_all[b, which]
        in_maps.append({"vt": vt_np, "esc": esc, "ph": ph})
    return in_maps


def kernel(alpha_real, alpha_imag, evals, evecs):
    from concourse import bass_utils

    nc = _get_module()

    in_maps = _make_in_maps(alpha_real, alpha_imag, evals, evecs)

    res = bass_utils.run_bass_kernel_spmd(
        nc, in_maps, core_ids=list(range(NCORES))
    )

    out = np.empty((B, N, N), np.complex64)
    for c in range(NCORES):
        outr = res.results[c]["outr"]
        outi = res.results[c]["outi"]
        for a in range(APC):
            b = c * APC + a
            out.real[b] = outr[a]
            out.imag[b] = outi[a]
    return out



# revision 13
# speedup vs baseline: 42.3913x; 42.3913x over previous
"""Trainium2 Bass kernel for batched displacement-operator construction.

Math: for each alpha_b,
    Da[b] = diag(u) @ (V @ diag(exp(-i r lam)) @ V.T) @ diag(v)
with u_i = w^i, v_j = w^-j, w = i*alpha/|alpha|.

Parity reduction: the generator H (tridiagonal) anticommutes with the
parity operator Pi = diag((-1)^n), so M = V exp(-i r Lam) V^T is real on
even i-j and purely imaginary on odd i-j, and eigenpairs come in
(lam, v), (-lam, Pi v) pairs.  Writing Ue/Uo for the even/odd rows of
the positive-lambda eigenvectors (512x512 each):

    M[2i',2j']     = Cee = Ue diag(2 cos r lam+) Ue^T      (real)
    M[2i'+1,2j'+1] = Coo = Uo diag(2 cos r lam+) Uo^T      (real)
    M[2i',2j'+1]   = i*Seo,  Seo = Ue diag(-2 sin r lam+) Uo^T
    M[2i'+1,2j']   = i*Soe,  Soe = Seo^T

so the two full 1024^3 real matmuls of the direct method collapse to
three 512^3 matmuls + transposes (4x fewer MACs).  The outer phase
w^(i-j) is Toeplitz; per parity block it is applied as an elementwise
multiply by a [128, 896] shifted-window table (host-precomputed), with
the even/odd column interleave done on-chip via stride-2 writes.
Outputs are bf16 (rel err ~2e-3, far under the 2e-2 gate), halving the
output DMA traffic.

Sharding: 16 alphas data-parallel over 8 cores (2 per core); Ue/Uo
replicated.  Both alphas share each LDWEIGHTS (4 moving streams per
stationary block).  Measured per-core body time ~23us (PE-roofline:
96 fp32r MMs ~20.5us + 32 transposes ~3.4us); DVE ~19us, ACT ~16us,
Pool ~4us, DMA-out 4.2MB ~15us all hide under PE.
"""

import sys

sys.path.insert(0, "/opt/trn_rl_repo")

import numpy as np

N = 1024
H = 512  # parity half-dimension
B = 16
NCORES = 8
APC = B // NCORES  # alphas per core
P = 128
HC = H // P  # chunks per half-dim (4)
NT = 512  # matmul free-dim / psum bank width (fp32)
TW = 896  # phase-window free size
OFF = 384  # phase-window offset: col t0 = OFF - 128*m

_cache = {}


def _build_module(reps=1, unroll=1):
    import contextlib

    import concourse.bacc as bacc
    import concourse.mybir as mybir
    import concourse.tile as tile

    f32 = mybir.dt.float32
    f32r = mybir.dt.float32r
    bf16 = mybir.dt.bfloat16

    nc = bacc.Bacc(
        "TRN2",
        target_bir_lowering=False,
        debug=False,
        num_devices=NCORES,
    )

    uet_d = nc.dram_tensor("uet", [H, H], f32, kind="ExternalInput")
    uot_d = nc.dram_tensor("uot", [H, H], f32, kind="ExternalInput")
    esc_d = nc.dram_tensor("esc", [P, APC * 2 * HC], f32, kind="ExternalInput")
    ph_d = nc.dram_tensor("ph", [P, APC * 6 * TW], bf16, kind="ExternalInput")
    outr_d = nc.dram_tensor("outr", [APC, N, N], bf16, kind="ExternalOutput")
    outi_d = nc.dram_tensor("outi", [APC, N, N], bf16, kind="ExternalOutput")

    with tile.TileContext(nc) as tc:
        with (
            tc.tile_pool(name="const", bufs=1) as cpool,
            tc.tile_pool(name="wts", bufs=2) as wpool,
            tc.tile_pool(name="work", bufs=2) as work,
            tc.tile_pool(name="outp", bufs=3) as outp,
            tc.tile_pool(name="ssbp", bufs=2) as ssbp,
            tc.tile_pool(name="psum", bufs=8, space="PSUM") as pp,
        ):
            esc = cpool.tile([P, APC * 2 * HC], f32)
            ph = cpool.tile([P, APC * 6 * TW], bf16)
            from concourse.masks import make_identity

            ident_f = cpool.tile([P, P], f32, name="ident_f")
            make_identity(nc, ident_f)

            uet = [
                cpool.tile([P, H], f32r, tag=f"uet{kc}", name=f"uet{kc}")
                for kc in range(HC)
            ]
            uot = [
                cpool.tile([P, H], f32r, tag=f"uot{kc}", name=f"uot{kc}")
                for kc in range(HC)
            ]

            nc.gpsimd.dma_start(esc[:], esc_d[:])
            nc.gpsimd.dma_start(ph[:], ph_d[:])
            # fp32r DRAM binding crashes the exec unit; DMA fp32 and cast.
            for kc in range(HC):
                tmp = work.tile([P, H], f32, tag="uin")
                nc.sync.dma_start(tmp[:], uet_d[kc * P : (kc + 1) * P, :])
                nc.vector.tensor_copy(uet[kc][:], tmp[:])
            for kc in range(HC):
                tmp = work.tile([P, H], f32, tag="uin")
                nc.sync.dma_start(tmp[:], uot_d[kc * P : (kc + 1) * P, :])
                nc.vector.tensor_copy(uot[kc][:], tmp[:])

            rep_ctx = (
                tc.For_i(0, reps, 1) if reps > 1 else contextlib.nullcontext()
            )
            with rep_ctx:
                for _u in range(unroll):
                    _emit_body(nc, tc, uet, uot, esc, ph, wpool, work, outp,
                               pp, outr_d, outi_d, mybir, ident_f, ssbp)

    nc.compile()
    return nc


def _emit_body(nc, tc, uet, uot, esc, ph, wpool, work, outp, pp,
               outr_d, outi_d, mybir, ident, ssbp):
    f32 = mybir.dt.float32
    f32r = mybir.dt.float32r
    bf16 = mybir.dt.bfloat16
    Alu = mybir.AluOpType
    Act = mybir.ActivationFunctionType

    # Moving operands for both alphas: diag-scaled rows of Ue^T / Uo^T.
    # lce = diag(2cos) Ue^T, lso = diag(-2sin) Uo^T, lco = diag(2cos) Uo^T
    lce = [[None] * HC for _ in range(APC)]
    lso = [[None] * HC for _ in range(APC)]
    lco = [[None] * HC for _ in range(APC)]
    for a in range(APC):
        for kc in range(HC):
            col_c = a * 2 * HC + kc
            col_s = a * 2 * HC + HC + kc
            tc_ = wpool.tile([P, H], f32r, tag=f"lce{kc}a{a}", name=f"lce{kc}a{a}")
            ts_ = wpool.tile([P, H], f32r, tag=f"lso{kc}a{a}", name=f"lso{kc}a{a}")
            to_ = wpool.tile([P, H], f32r, tag=f"lco{kc}a{a}", name=f"lco{kc}a{a}")
            nc.scalar.activation(tc_[:], uet[kc][:], Act.Copy,
                                 scale=esc[:, col_c : col_c + 1])
            nc.scalar.activation(ts_[:], uot[kc][:], Act.Copy,
                                 scale=esc[:, col_s : col_s + 1])
            nc.scalar.activation(to_[:], uot[kc][:], Act.Copy,
                                 scale=esc[:, col_c : col_c + 1])
            lce[a][kc] = tc_
            lso[a][kc] = ts_
            lco[a][kc] = to_

    def win(a, wi, m):
        t0 = a * 6 * TW + wi * TW + (OFF - P * m)
        return ph[:, t0 : t0 + NT]

    ssb = {}
    # Even output rows 2*(m*128+p): even cols from Cee, odd from Seo.
    # Both alphas share each LDWEIGHTS: 4 moving streams per stationary.
    for m in range(HC):
        pcee = [pp.tile([P, NT], f32, tag="ps", name=f"pcee{m}_{_a}")
                for _a in range(APC)]
        pseo = [pp.tile([P, NT], f32, tag="ps", name=f"pseo{m}_{_a}")
                for _a in range(APC)]
        for kc in range(HC):
            wap = uet[kc][:, m * P : (m + 1) * P]
            st = kc == 0
            sp = kc == HC - 1
            for a in range(APC):
                nc.tensor.matmul(pcee[a][:], wap, lce[a][kc][:],
                                 start=st, stop=sp)
                nc.tensor.matmul(pseo[a][:], wap, lso[a][kc][:],
                                 start=st, stop=sp)
        for a in range(APC):
            # SBUF copy of Seo chunk: transpose source for Soe AND the
            # Pool-readable operand for the odd-column phase mults (Pool
            # has no PSUM port).
            sb = ssbp.tile([P, NT], f32, tag=f"ssb{m}a{a}", name=f"ssb{m}a{a}")
            nc.scalar.activation(sb[:], pseo[a][:], Act.Copy)
            ssb[a, m] = sb

            ere = outp.tile([P, N], bf16, tag="ere")
            eim = outp.tile([P, N], bf16, tag="eim")
            nc.vector.tensor_tensor(ere[:, 0:N:2], pcee[a][:], win(a, 0, m),
                                    Alu.mult)
            nc.vector.tensor_tensor(eim[:, 0:N:2], pcee[a][:], win(a, 1, m),
                                    Alu.mult)
            nc.gpsimd.tensor_tensor(ere[:, 1:N:2], sb[:], win(a, 2, m),
                                    Alu.mult)
            nc.gpsimd.tensor_tensor(eim[:, 1:N:2], sb[:], win(a, 3, m),
                                    Alu.mult)
            nc.sync.dma_start(
                outr_d[a, 2 * m * P : 2 * (m + 1) * P : 2, :], ere[:]
            )
            nc.sync.dma_start(
                outi_d[a, 2 * m * P : 2 * (m + 1) * P : 2, :], eim[:]
            )

    # Odd output rows: odd cols from Coo, even cols from Soe = Seo^T.
    for m in range(HC):
        pcoo = [pp.tile([P, NT], f32, tag="ps", name=f"pcoo{m}_{_a}")
                for _a in range(APC)]
        psoe = [pp.tile([P, NT], f32, tag="ps", name=f"psoe{m}_{_a}")
                for _a in range(APC)]
        for kc in range(HC):
            wap = uot[kc][:, m * P : (m + 1) * P]
            st = kc == 0
            sp = kc == HC - 1
            for a in range(APC):
                nc.tensor.matmul(pcoo[a][:], wap, lco[a][kc][:],
                                 start=st, stop=sp)
        for a in range(APC):
            for q in range(HC):
                nc.tensor.matmul(
                    psoe[a][:, q * P : (q + 1) * P],
                    ssb[a, q][:, m * P : (m + 1) * P], ident[:],
                    is_transpose=True, start=True, stop=True,
                )
            ore = outp.tile([P, N], bf16, tag="ore")
            oim = outp.tile([P, N], bf16, tag="oim")
            nc.vector.tensor_tensor(ore[:, 1:N:2], pcoo[a][:], win(a, 0, m),
                                    Alu.mult)
            nc.vector.tensor_tensor(oim[:, 1:N:2], pcoo[a][:], win(a, 1, m),
                                    Alu.mult)
            nc.vector.tensor_tensor(ore[:, 0:N:2], psoe[a][:], win(a, 4, m),
                                    Alu.mult)
            nc.vector.tensor_tensor(oim[:, 0:N:2], psoe[a][:], win(a, 5, m),
                                    Alu.mult)
            nc.sync.dma_start(
                outr_d[a, 2 * m * P + 1 : 2 * (m + 1) * P : 2, :], ore[:]
            )
            nc.sync.dma_start(
                outi_d[a, 2 * m * P + 1 : 2 * (m + 1) * P : 2, :], oim[:]
            )


def _get_module():
    if "nc" not in _cache:
        _cache["nc"] = _build_module()
    return _cache["nc"]


def _host_precompute(alpha_real, alpha_imag, evals):
    """Per-alpha scalar/window tables, fp64 host math for the phases."""
    ar = np.asarray(alpha_real, np.float32)
    ai = np.asarray(alpha_imag, np.float32)
    ev = np.asarray(evals, np.float64)
    lamp = ev[H:]  # positive eigenvalues, ascending

    esc_all = np.empty((B, 2, HC, P), np.float32)  # (b, c2/s2, kc, p)
    ph_all = np.empty((B, 6, P, TW), np.float32)  # (b, window, p, t)

    prow = np.arange(P)[:, None]
    tcol = np.arange(TW)[None, :]
    idx = prow - tcol + OFF + (H - 1)  # into d-tables of length 2H-1
    d = np.arange(-(H - 1), H).astype(np.float64)

    for b in range(B):
        alpha = complex(float(ar[b]), float(ai[b]))
        r = np.float64(abs(np.complex64(alpha))) + np.float64(np.float32(1e-10))
        w = 1j * alpha / r

        c2 = (2.0 * np.cos(r * lamp)).astype(np.float32)
        s2 = (-2.0 * np.sin(r * lamp)).astype(np.float32)
        esc_all[b, 0] = c2.reshape(HC, P)
        esc_all[b, 1] = s2.reshape(HC, P)

        w2d = w ** (2 * d)
        w2dm = w ** (2 * d - 1)
        w2dp = w ** (2 * d + 1)
        tabs = (
            np.real(w2d), np.imag(w2d),
            -np.imag(w2dm), np.real(w2dm),
            -np.imag(w2dp), np.real(w2dp),
        )
        for wi, tab in enumerate(tabs):
            ph_all[b, wi] = tab.astype(np.float32)[idx]

    return esc_all, ph_all


def _make_in_maps(alpha_real, alpha_imag, evals, evecs):
    evecs_f = np.asarray(evecs, np.float32)
    uet_np = np.ascontiguousarray(evecs_f[0::2, H:].T)  # [k, i'] even sites
    uot_np = np.ascontiguousarray(evecs_f[1::2, H:].T)
    esc_all, ph_all = _host_precompute(alpha_real, alpha_imag, evals)

    in_maps = []
    for c in range(NCORES):
        bs = [c * APC + a for a in range(APC)]
        esc = np.empty((P, APC * 2 * HC), np.float32)
        import ml_dtypes
        ph = np.empty((P, APC * 6 * TW), ml_dtypes.bfloat16)
        for a, b in enumerate(bs):
            for which in range(2):
                cols = a * 2 * HC + which * HC
                esc[:, cols : cols + HC] = esc_all[b, which].T
            for wi in range(6):
                wbase = (a * 6 + wi) * TW
                ph[:, wbase : wbase + TW] = ph_all[b, wi]
        in_maps.append({"uet": uet_np, "uot": uot_np, "esc": esc, "ph": ph})
    return in_maps


def kernel(alpha_real, alpha_imag, evals, evecs):
    from concourse import bass_utils

    nc = _get_module()

    in_maps = _make_in_maps(alpha_real, alpha_imag, evals, evecs)

    res = bass_utils.run_bass_kernel_spmd(
        nc, in_maps, core_ids=list(range(NCORES))
    )

    out = np.empty((B, N, N), np.complex64)
    for c in range(NCORES):
        outr = np.asarray(res.results[c]["outr"], dtype=np.float32)
        outi = np.asarray(res.results[c]["outi"], dtype=np.float32)
        for a in range(APC):
            b = c * APC + a
            out.real[b] = outr[a]
            out.imag[b] = outi[a]
    return out
